# revision 11
# baseline (speedup 1.0000x reference)
"""Trainium2 Bass kernel for nn_MultiMPNN (gnn_message_passing).

Reference computation (B=4, N=512, Z=64, E=16, H=128):
    msgs[b,i,j,:] = z[b,i]@W_i + z[b,j]@W_j + e_feat[b,i,j]@W_e + b_msg
    agg[b,i,:]    = max_j (msgs + (adj>0 ? 0 : -inf))
    out           = z@Wu_z + agg@Wu_h + b_upd

Sharding: 8 cores = (batch b, half of destination rows i).  Each core owns
256 i-rows and the full j axis.

Device-side design (v2):
 1. Everything under the max folds into ONE fp8 DoubleRow matmul per row
    with augmented contraction K = E + Z = 80 packed as [40, 2]:
      lhsT[40,2,128] = [W_e ; W_j]              (constant, e4m3)
      rhs [40,2,w]   = [e_feat[b,i,sel].T ; z[b,sel].T]  (streamed, e4m3)
      PSUM[h,j] = ze + zj   ->  per-row max over j -> agg column
    No mask lane: padding slots replicate the row's first ACTIVE column, so
    the max is unchanged.  zi + b_msg commute out of the max and fold into
    the final linear, whose z@Wu_z part is computed on the host (tiny, f32).
 2. The host compacts the j axis per row (only j with adj=1 participate),
    with a uniform width per 32-row block of count-sorted rows.
 3. The max-reduce of PSUM [128, w] per row is balanced over two engine
    paths per 4-row quad (one row per PSUM bank, 4 banks per quad).  Only
    DVE and ACT can read PSUM (one operand per instruction); GPSIMD is
    SBUF-only and its TensorTensor ucode has no max op, so:
      d) DVE reduce_max straight from PSUM (one instruction, done).
      a) ACT copy PSUM->SBUF bf16; adjacent a-quads share one staging
         tile so ONE DVE 4x-mode reduce_max covers 8 rows.
    Block pattern d,a,a,d,a,a,d,a balances DVE and ACT at ~165 ns/row.
"""

import numpy as np
import ml_dtypes

import concourse.bacc as bacc
import concourse.mybir as mybir
import concourse.tile as tile
from concourse import bass_utils
from concourse.bass_interp import get_hw_module
from contextlib import ExitStack

B, N, Z, E, H = 4, 512, 64, 16, 128
NCORES = 8
IH = N * B // NCORES          # 256 destination rows per core
KAUG = E + Z                  # 80
KP = KAUG // 2                # 40 partitions, 2 k-tiles (DoubleRow)
BANK = 512                    # f32 elems per PSUM bank
QB = 4                        # rows (banks) per reduce quad
# per-quad reduce paths: d=DVE-direct, a=ACT-copy + DVE 4x reduce
# (adjacent 'a' pairs share one staging tile).  8-quad block pattern;
# 2-quad ramp blocks.
PAT_A = "daadaada"
PAT_RAMP = "da"

F32 = mybir.dt.float32
BF16 = mybir.dt.bfloat16
FP8 = mybir.dt.float8e4
NP_FP8 = ml_dtypes.float8_e4m3
NP_BF16 = ml_dtypes.bfloat16

# ramp-up: small first blocks so the PE starts early; 32-row steady blocks
SIZES = [8, 8, 8, 8] + [32] * 7
assert sum(SIZES) == IH

TRACE = False                 # test.py sets True to capture an NTFF profile
TRACE_DIR = None              # optional fixed dir for trace artifacts
LAST_RESULTS = None           # BassKernelResults of the last run (for test.py)

_MODULE_CACHE = {}


def _ensure_ntff_hook():
    """The agent image's antenv lacks axon_hooks; recreate it so
    run_bass_kernel_spmd(trace=True) can reach the axon NTFF profiler."""
    import sys
    import types

    try:
        import antenv.axon_hooks  # noqa: F401

        return
    except ImportError:
        pass
    import antenv
    from trn_agent_boot.trn_boot import _ntff_profile_via_ctypes

    state = {"h": _ntff_profile_via_ctypes("/opt/axon/libaxon_pjrt.so")}
    mod = types.ModuleType("antenv.axon_hooks")
    mod.get_axon_ntff_profile_hook = lambda: state["h"]
    mod.set_axon_ntff_profile_hook = lambda h: state.__setitem__("h", h)
    sys.modules["antenv.axon_hooks"] = mod
    antenv.axon_hooks = mod


def _build_module(widths):
    widths = list(widths)
    offs = [0]
    for gsz, w in zip(SIZES, widths):
        offs.append(offs[-1] + gsz * w)
    tot = offs[-1]

    nc = bacc.Bacc(
        "TRN2",
        target_bir_lowering=False,
        debug=False,
        enable_asserts=False,
        num_devices=NCORES,
    )

    stream = nc.dram_tensor("stream", [KP, 2 * tot], FP8, kind="ExternalInput")
    lhst = nc.dram_tensor("lhst", [KP, 2 * H], FP8, kind="ExternalInput")
    zit = nc.dram_tensor("zit", [H, IH], F32, kind="ExternalInput")
    hostc = nc.dram_tensor("hostc", [H, IH], F32, kind="ExternalInput")
    wuh = nc.dram_tensor("wuh", [H, H], F32, kind="ExternalInput")
    ident = nc.dram_tensor("ident", [H, H], F32, kind="ExternalInput")
    out = nc.dram_tensor("out", [IH, H], F32, kind="ExternalOutput")

    with ExitStack() as ctx:
        tc = ctx.enter_context(tile.TileContext(nc))
        const = ctx.enter_context(tc.tile_pool(name="const", bufs=1))
        mega = ctx.enter_context(tc.tile_pool(name="mega", bufs=4))
        psum = ctx.enter_context(tc.tile_pool(name="psum", bufs=2, space="PSUM"))
        stage = ctx.enter_context(tc.tile_pool(name="stage", bufs=6))

        lhst_sb = const.tile([KP, 2 * H], FP8, tag="lhst")
        nc.sync.dma_start(lhst_sb[:, :], lhst.ap())
        lhst3 = lhst_sb[:, :].rearrange("p (s m) -> p s m", s=2)
        zit_sb = const.tile([H, IH], F32, tag="zit")
        nc.sync.dma_start(zit_sb[:, :], zit.ap())
        hostc_sb = const.tile([H, IH], F32, tag="hostc")
        nc.sync.dma_start(hostc_sb[:, :], hostc.ap())
        wuh_sb = const.tile([H, H], F32, tag="wuh")
        nc.sync.dma_start(wuh_sb[:, :], wuh.ap())
        ident_sb = const.tile([H, H], F32, tag="ident")
        nc.sync.dma_start(ident_sb[:, :], ident.ap())

        magg = const.tile([H, IH], BF16, tag="magg")

        # PE warm-up: the HAM clock gate keeps the PE at 1.2 GHz until it has
        # been busy for ~4 us.  The PE is idle during the DMA-dominated
        # startup window anyway, so burn it with dummy matmuls on a zeroed
        # scratch tile to reach 2.4 GHz before the real work arrives.
        warm_a = const.tile([H, BANK], BF16, tag="warm_a")
        nc.vector.memset(warm_a[:, :], 0.0)
        pw = psum.tile([H, QB * BANK], F32, tag="ps")
        for _ in range(6):
            nc.tensor.matmul(
                pw[:, :BANK], warm_a[:, :H], warm_a[:, :], start=True, stop=True
            )

        stream_ap = stream.ap().rearrange("p (s t) -> p s t", s=2)

        qidx = 0
        row0 = 0
        for blk, (gsz, w) in enumerate(zip(SIZES, widths)):
            span = gsz * w
            off = offs[blk]
            mb = mega.tile([KP, 2 * span], FP8, tag="mega")
            mb3 = mb[:, :].rearrange("p (s t) -> p s t", s=2)
            nc.sync.dma_start(mb3[:, :, :], stream_ap[:, :, off : off + span])

            pat = PAT_RAMP if gsz < 32 else PAT_A
            nq = gsz // QB
            pend = None  # first half of an a-pair: (sv tile, i0)
            for q in range(nq):
                ps = psum.tile([H, QB * BANK], F32, tag="ps")
                ps3 = ps[:, :].rearrange("p (b j) -> p b j", b=QB)
                for r in range(QB):
                    c0 = (q * QB + r) * w
                    nc.tensor.matmul(
                        ps3[:, r, :w],
                        lhst3[:, :, :],
                        mb3[:, :, c0 : c0 + w],
                        start=True,
                        stop=True,
                        perf_mode=mybir.MatmulPerfMode.DoubleRow,
                    )
                i0 = row0 + q * QB
                path = pat[q % len(pat)]
                qidx += 1
                if path == "d":
                    nc.vector.reduce_max(
                        magg[:, i0 : i0 + QB], ps3[:, :, :w],
                        axis=mybir.AxisListType.X,
                    )
                    continue
                nxt = pat[(q + 1) % len(pat)] if q + 1 < nq else "d"
                if pend is None and nxt == "a":
                    sv = stage.tile([H, 2 * QB * w], BF16, tag="apair")
                    sv4 = sv[:, :].rearrange("p (b j) -> p b j", b=2 * QB)
                    nc.scalar.copy(sv4[:, :QB, :], ps3[:, :, :w])
                    pend = (sv, i0)
                elif pend is not None:
                    sv, ip = pend
                    pend = None
                    assert ip + QB == i0
                    sv4 = sv[:, :].rearrange("p (b j) -> p b j", b=2 * QB)
                    nc.scalar.copy(sv4[:, QB:, :], ps3[:, :, :w])
                    nc.vector.reduce_max(
                        magg[:, ip : ip + 2 * QB], sv4[:, :, :],
                        axis=mybir.AxisListType.X,
                    )
                else:
                    sv = stage.tile([H, QB * w], BF16, tag="asing")
                    sv3 = sv[:, :].rearrange("p (b j) -> p b j", b=QB)
                    nc.scalar.copy(sv3[:, :, :], ps3[:, :, :w])
                    nc.vector.reduce_max(
                        magg[:, i0 : i0 + QB], sv3[:, :, :],
                        axis=mybir.AxisListType.X,
                    )
            if blk in (3, 4, 5):
                for _ in range(8):
                    nc.tensor.matmul(
                        pw[:, :BANK],
                        warm_a[:, :H],
                        warm_a[:, :],
                        start=True,
                        stop=True,
                    )
            row0 += gsz

        aggt = const.tile([H, IH], F32, tag="aggt")
        nc.vector.tensor_add(aggt[:, :], magg[:, :], zit_sb[:, :])

        psf = psum.tile([H, QB * BANK], F32, tag="ps")
        nc.tensor.matmul(psf[:, :IH], wuh_sb[:, :], aggt[:, :], start=True, stop=True)

        outt = const.tile([H, IH], F32, tag="outt")
        nc.vector.tensor_add(outt[:, :], psf[:, :IH], hostc_sb[:, :])

        out_ap = out.ap()
        for t in range(IH // H):
            pst = psum.tile([H, QB * BANK], F32, tag="ps")
            nc.tensor.transpose(
                pst[:, :H], outt[:, t * H : (t + 1) * H], ident_sb[:, :]
            )
            osb = const.tile([H, H], F32, tag=f"osb{t}")
            nc.scalar.copy(osb[:, :], pst[:, :H])
            nc.sync.dma_start(out_ap[t * H : (t + 1) * H, :], osb[:, :])

    nc.compile()
    nc.m = get_hw_module(nc.m)
    return nc


def _prepare(z, e_feat, adj, W_msg, b_msg, W_upd, b_upd):
    """Host-side sharding + compaction with per-block uniform widths.

    Rows are sorted by active-edge count (descending) so each block of
    consecutive rows gets a tight shared width.  Padding slots replicate the
    row's first active column (max-neutral).  Returns (in_maps, widths,
    orders); out rows come back permuted by `orders`.
    """
    W_i, W_j, W_e = W_msg[:Z], W_msg[Z : 2 * Z], W_msg[2 * Z :]
    Wu_z, Wu_h = W_upd[:Z], W_upd[Z:]

    counts = (adj > 0).sum(axis=-1)                   # [B, N]
    orders, csort = [], []
    for c in range(NCORES):
        b, half = divmod(c, NCORES // B)
        cnt = counts[b, half * IH : (half + 1) * IH]
        order = np.argsort(-cnt, kind="stable")
        orders.append(order)
        csort.append(cnt[order])
    csort = np.stack(csort)                           # [NCORES, IH]

    widths = []
    row0 = 0
    for gsz in SIZES:
        wmax = int(csort[:, row0 : row0 + gsz].max())
        widths.append(min(N, (wmax + 7) // 8 * 8))
        row0 += gsz
    widths = np.array(widths, dtype=int)
    row_w = np.repeat(widths, SIZES)                  # [IH]
    offs = np.concatenate([[0], np.cumsum(SIZES * widths)])
    tot = int(offs[-1])
    maxw = int(widths.max())

    # lhsT [40, 2, 128]: k = s*40 + p -> W_aug[k] rows
    W_aug = np.concatenate([W_e, W_j], axis=0).astype(NP_FP8)   # [80, H]
    lhst_np = np.ascontiguousarray(
        W_aug.reshape(2, KP, H).transpose(1, 0, 2).reshape(KP, 2 * H)
    )
    wuh_np = np.ascontiguousarray(Wu_h, np.float32)
    ident_np = np.eye(H, dtype=np.float32)

    in_maps = []
    for c in range(NCORES):
        b, half = divmod(c, NCORES // B)
        sl = slice(half * IH, (half + 1) * IH)
        order = orders[c]
        adj_blk = (adj[b, sl] > 0)[order]             # [IH, N] sorted rows
        jorder = np.argsort(~adj_blk, axis=-1, kind="stable")[:, :maxw]
        pad = ~np.take_along_axis(adj_blk, jorder, axis=1)
        # padding slots replicate the first (active) column of the row
        jorder = np.where(pad, jorder[:, :1], jorder)
        e_sel = np.take_along_axis(
            e_feat[b, sl][order], jorder[:, :, None], axis=1
        )                                             # [IH, maxw, E]
        z_sel = z[b][jorder]                          # [IH, maxw, Z]
        aug8 = np.concatenate(
            [e_sel.astype(NP_FP8), z_sel.astype(NP_FP8)], axis=-1
        )                                             # [IH, maxw, 80]

        stream = np.empty((KP, 2, tot), dtype=NP_FP8)
        row0 = 0
        for blk, (gsz, w) in enumerate(zip(SIZES, widths)):
            blkv = aug8[row0 : row0 + gsz, :w, :]     # [gsz, w, 80]
            # [80, gsz*w] -> [2, 40, span]; k = s*40 + p
            kv = blkv.transpose(2, 0, 1).reshape(KAUG, gsz * w)
            stream[:, :, offs[blk] : offs[blk + 1]] = (
                kv.reshape(2, KP, gsz * w).transpose(1, 0, 2)
            )
            row0 += gsz

        zperm = z[b, sl][order]
        in_maps.append(
            {
                "stream": np.ascontiguousarray(stream.reshape(KP, 2 * tot)),
                "lhst": lhst_np,
                "zit": np.ascontiguousarray(
                    (zperm @ W_i).T + b_msg[:, None], dtype=np.float32
                ),
                "hostc": np.ascontiguousarray(
                    (zperm @ Wu_z + b_upd).T, dtype=np.float32
                ),
                "wuh": wuh_np,
                "ident": ident_np,
            }
        )
    return in_maps, widths, orders


def kernel(z, e_feat, adj, W_msg, b_msg, W_upd, b_upd):
    global LAST_RESULTS

    z = np.asarray(z, np.float32)
    e_feat = np.asarray(e_feat, np.float32)
    adj = np.asarray(adj)
    W_msg = np.asarray(W_msg, np.float32)
    b_msg = np.asarray(b_msg, np.float32)
    W_upd = np.asarray(W_upd, np.float32)
    b_upd = np.asarray(b_upd, np.float32)

    in_maps, widths, orders = _prepare(z, e_feat, adj, W_msg, b_msg, W_upd, b_upd)

    key = tuple(widths)
    if key not in _MODULE_CACHE:
        _MODULE_CACHE[key] = _build_module(widths)
    nc = _MODULE_CACHE[key]

    if TRACE:
        _ensure_ntff_hook()
    res = bass_utils.run_bass_kernel_spmd(
        nc, in_maps, core_ids=list(range(NCORES)), trace=TRACE, tmpdir=TRACE_DIR
    )
    LAST_RESULTS = res

    full = np.empty((B, N, H), np.float32)
    for c in range(NCORES):
        b, half = divmod(c, NCORES // B)
        full[b, half * IH + orders[c]] = res.results[c]["out"]
    return full


if __name__ == "__main__":
    rng = np.random.default_rng(0)
    ins = {
        "z": rng.standard_normal((B, N, Z)).astype(np.float32),
        "e_feat": rng.standard_normal((B, N, N, E)).astype(np.float32),
        "adj": (rng.random((B, N, N)) < 0.5).astype(np.int32),
        "W_msg": (rng.standard_normal((2 * Z + E, H)) * 0.1).astype(np.float32),
        "b_msg": np.zeros(H, np.float32),
        "W_upd": (rng.standard_normal((Z + H, H)) * 0.1).astype(np.float32),
        "b_upd": np.zeros(H, np.float32),
    }
    out = kernel(**ins)
    print("out", out.shape, out.dtype, float(np.abs(out).max()))


# revision 13
# speedup vs baseline: 1.0450x; 1.0450x over previous
"""Trainium2 Bass kernel for nn_MultiMPNN (gnn_message_passing).

Reference computation (B=4, N=512, Z=64, E=16, H=128):
    msgs[b,i,j,:] = z[b,i]@W_i + z[b,j]@W_j + e_feat[b,i,j]@W_e + b_msg
    agg[b,i,:]    = max_j (msgs + (adj>0 ? 0 : -inf))
    out           = z@Wu_z + agg@Wu_h + b_upd

Sharding: 8 cores = (batch b, half of destination rows i).  Each core owns
256 i-rows and the full j axis.

Device-side design (v3), tuned to measured TRN2 rates:
  PE matmul 1 col/cycle (ldweights hidden when back-to-back),
  DVE reduce 1.04 ns/elem (no fast mode), DVE tensor_tensor bf16 0.52
  ns/out (2x), ACT copy ~0.96 ns/elem, GPSIMD has no max op.

 1. One fp8 matmul per row folds everything under the max, augmented
    contraction K = E + Z = 80:
      lhsT[80,128] = [W_e ; W_j]  (const, e4m3)
      rhs [80,w]   = [e_feat[b,i,sel].T ; z[b,sel].T]  (streamed, e4m3)
    No mask lane: padding slots replicate the row's first ACTIVE column.
    zi + b_msg commute out of the max into the final linear (z@Wu_z and
    z@W_i computed on host, exact f32).
 2. Host compacts the j axis per row (only adj=1 columns), width uniform
    per block of count-sorted rows.
 3. PSUM drain per 4-row quad (row per bank), pattern a,a,d,a,a,d,a,a:
      d) DVE reduce_max straight from PSUM (1.04/elem, done in one).
      a) ACT copy PSUM->SBUF bf16; pairs of quads share a staging tile,
         then a DVE tensor_tensor(max) halving chain (w/2 -> w/4 -> w/8,
         0.26/elem consumed) + one small reduce_max finishes 8 rows.
 4. Filler matmuls into the unused tail columns of each quad's banks keep
    the PE p-state at 2.4 GHz (PE has slack; the drains are the
    bottleneck, and a cold PE would become one).
 5. The final linear runs in two i-halves so the first half overlaps the
    last quads.
"""

import numpy as np
import ml_dtypes

import concourse.bacc as bacc
import concourse.mybir as mybir
import concourse.tile as tile
from concourse import bass_utils
from concourse.bass_interp import get_hw_module
from contextlib import ExitStack

B, N, Z, E, H = 4, 512, 64, 16, 128
NCORES = 8
IH = N * B // NCORES          # 256 destination rows per core
KAUG = E + Z                  # 80
BANK = 512                    # f32 elems per PSUM bank
QB = 4                        # rows (banks) per quad

F32 = mybir.dt.float32
BF16 = mybir.dt.bfloat16
FP8 = mybir.dt.float8e4
NP_FP8 = ml_dtypes.float8_e4m3
NP_BF16 = ml_dtypes.bfloat16

SIZES = [8, 8, 8, 8] + [32] * 7
assert sum(SIZES) == IH
PAT_A = "aadaadaa"            # per-quad paths in a 32-row block
PAT_RAMP = "da"               # 8-row ramp blocks
FILLERS = 2                   # PE p-state keeper matmuls per quad

TRACE = False
TRACE_DIR = None
LAST_RESULTS = None

_MODULE_CACHE = {}


def _ensure_ntff_hook():
    """The agent image's antenv lacks axon_hooks; recreate it so
    run_bass_kernel_spmd(trace=True) can reach the axon NTFF profiler."""
    import sys
    import types

    try:
        import antenv.axon_hooks  # noqa: F401

        return
    except ImportError:
        pass
    import antenv
    from trn_agent_boot.trn_boot import _ntff_profile_via_ctypes

    state = {"h": _ntff_profile_via_ctypes("/opt/axon/libaxon_pjrt.so")}
    mod = types.ModuleType("antenv.axon_hooks")
    mod.get_axon_ntff_profile_hook = lambda: state["h"]
    mod.set_axon_ntff_profile_hook = lambda h: state.__setitem__("h", h)
    sys.modules["antenv.axon_hooks"] = mod
    antenv.axon_hooks = mod


def _build_module(widths):
    widths = list(widths)
    offs = [0]
    for gsz, w in zip(SIZES, widths):
        offs.append(offs[-1] + gsz * w)
    tot = offs[-1]

    nc = bacc.Bacc(
        "TRN2",
        target_bir_lowering=False,
        debug=False,
        enable_asserts=False,
        num_devices=NCORES,
    )

    stream = nc.dram_tensor("stream", [KAUG, tot], FP8, kind="ExternalInput")
    lhst = nc.dram_tensor("lhst", [KAUG, H], FP8, kind="ExternalInput")
    zit = nc.dram_tensor("zit", [H, IH], F32, kind="ExternalInput")
    hostc = nc.dram_tensor("hostc", [H, IH], F32, kind="ExternalInput")
    wuh = nc.dram_tensor("wuh", [H, H], F32, kind="ExternalInput")
    ident = nc.dram_tensor("ident", [H, H], F32, kind="ExternalInput")
    out = nc.dram_tensor("out", [IH, H], F32, kind="ExternalOutput")

    with ExitStack() as ctx:
        tc = ctx.enter_context(tile.TileContext(nc))
        const = ctx.enter_context(tc.tile_pool(name="const", bufs=1))
        mega = ctx.enter_context(tc.tile_pool(name="mega", bufs=4))
        psum = ctx.enter_context(tc.tile_pool(name="psum", bufs=2, space="PSUM"))
        stage = ctx.enter_context(tc.tile_pool(name="stage", bufs=6))

        # stream block 0 + lhst first so the PE can start ASAP; big consts
        # go on the scalar DMA queue (needed only by the tail).
        stream_ap = stream.ap()
        mb0 = mega.tile([KAUG, SIZES[0] * widths[0]], FP8, tag="mega")
        nc.sync.dma_start(mb0[:, :], stream_ap[:, : offs[1]])
        lhst_sb = const.tile([KAUG, H], FP8, tag="lhst")
        nc.sync.dma_start(lhst_sb[:, :], lhst.ap())
        zit_sb = const.tile([H, IH], F32, tag="zit")
        nc.scalar.dma_start(zit_sb[:, :], zit.ap())
        hostc_sb = const.tile([H, IH], F32, tag="hostc")
        nc.scalar.dma_start(hostc_sb[:, :], hostc.ap())
        wuh_sb = const.tile([H, H], F32, tag="wuh")
        nc.scalar.dma_start(wuh_sb[:, :], wuh.ap())
        ident_sb = const.tile([H, H], F32, tag="ident")
        nc.scalar.dma_start(ident_sb[:, :], ident.ap())

        magg0 = const.tile([H, IH // 2], BF16, tag="magg0")
        magg1 = const.tile([H, IH // 2], BF16, tag="magg1")

        def magg_sl(i0, n):
            t, o = (magg0, i0) if i0 < IH // 2 else (magg1, i0 - IH // 2)
            return t[:, o : o + n]

        # PE warm-up: reach 2.4 GHz before the first real matmul.
        warm_a = const.tile([H, BANK], BF16, tag="warm_a")
        nc.vector.memset(warm_a[:, :], 0.0)
        pw = psum.tile([H, QB * BANK], F32, tag="ps")
        for _ in range(6):
            nc.tensor.matmul(
                pw[:, :BANK], warm_a[:, :H], warm_a[:, :], start=True, stop=True
            )

        def tail_half(t):
            """Final linear + transpose + store for i-columns [t*128, t*128+128)."""
            sl = slice(t * H, (t + 1) * H)
            aggt = const.tile([H, H], F32, tag=f"aggt{t}")
            mt = magg0 if t == 0 else magg1
            nc.vector.tensor_add(aggt[:, :], mt[:, :], zit_sb[:, sl])
            psf = psum.tile([H, QB * BANK], F32, tag="ps")
            nc.tensor.matmul(
                psf[:, :H], wuh_sb[:, :], aggt[:, :], start=True, stop=True
            )
            outt = const.tile([H, H], F32, tag=f"outt{t}")
            nc.vector.tensor_add(outt[:, :], psf[:, :H], hostc_sb[:, sl])
            pst = psum.tile([H, QB * BANK], F32, tag="ps")
            nc.tensor.transpose(pst[:, :H], outt[:, :], ident_sb[:, :])
            osb = const.tile([H, H], F32, tag=f"osb{t}")
            nc.scalar.copy(osb[:, :], pst[:, :H])
            nc.sync.dma_start(out.ap()[sl, :], osb[:, :])

        row0 = 0
        for blk, (gsz, w) in enumerate(zip(SIZES, widths)):
            span = gsz * w
            off = offs[blk]
            if blk == 0:
                mb = mb0
            else:
                mb = mega.tile([KAUG, span], FP8, tag="mega")
                nc.sync.dma_start(mb[:, :], stream_ap[:, off : off + span])

            w2, w4, w8 = w // 2, w // 4, w // 8
            ftail = BANK - w                      # free tail cols per bank
            pat = PAT_RAMP if gsz < 32 else PAT_A
            nq = gsz // QB
            pend = None
            for q in range(nq):
                ps = psum.tile([H, QB * BANK], F32, tag="ps")
                ps3 = ps[:, :].rearrange("p (b j) -> p b j", b=QB)
                for r in range(QB):
                    c0 = (q * QB + r) * w
                    nc.tensor.matmul(
                        ps3[:, r, :w],
                        lhst_sb[:, :],
                        mb[:, c0 : c0 + w],
                        start=True,
                        stop=True,
                    )
                # p-state keepers: write unused bank tails, no new deps
                for f in range(FILLERS if ftail >= 64 else 0):
                    nc.tensor.matmul(
                        ps3[:, f, w:BANK],
                        lhst_sb[:, :],
                        mb[:, :ftail],
                        start=True,
                        stop=True,
                    )
                i0 = row0 + q * QB
                path = pat[q % len(pat)]
                if path == "d":
                    nc.vector.reduce_max(
                        magg_sl(i0, QB), ps3[:, :, :w],
                        axis=mybir.AxisListType.X,
                    )
                    continue
                nxt = pat[(q + 1) % len(pat)] if q + 1 < nq else "d"
                if pend is None and nxt == "a":
                    sv = stage.tile([H, 2 * QB * w], BF16, tag="apair")
                    sv4 = sv[:, :].rearrange("p (b j) -> p b j", b=2 * QB)
                    nc.scalar.copy(sv4[:, :QB, :], ps3[:, :, :w])
                    pend = (sv, i0)
                elif pend is not None:
                    sv, ip = pend
                    pend = None
                    assert ip + QB == i0
                    sv4 = sv[:, :].rearrange("p (b j) -> p b j", b=2 * QB)
                    nc.scalar.copy(sv4[:, QB:, :], ps3[:, :, :w])
                    hv = stage.tile([H, 2 * QB * w2], BF16, tag="ahalf")
                    hv4 = hv[:, :].rearrange("p (b j) -> p b j", b=2 * QB)
                    nc.vector.tensor_tensor(
                        hv4[:, :, :], sv4[:, :, :w2], sv4[:, :, w2:w],
                        mybir.AluOpType.max,
                    )
                    qv = stage.tile([H, 2 * QB * w4], BF16, tag="aquar")
                    qv4 = qv[:, :].rearrange("p (b j) -> p b j", b=2 * QB)
                    nc.vector.tensor_tensor(
                        qv4[:, :, :], hv4[:, :, :w4], hv4[:, :, w4:w2],
                        mybir.AluOpType.max,
                    )
                    ov = stage.tile([H, 2 * QB * w8], BF16, tag="aeighth")
                    ov4 = ov[:, :].rearrange("p (b j) -> p b j", b=2 * QB)
                    nc.vector.tensor_tensor(
                        ov4[:, :, :], qv4[:, :, :w8], qv4[:, :, w8:w4],
                        mybir.AluOpType.max,
                    )
                    nc.vector.reduce_max(
                        magg_sl(ip, 2 * QB), ov4[:, :, :],
                        axis=mybir.AxisListType.X,
                    )
                else:  # single 'a' quad (ramp blocks)
                    sv = stage.tile([H, QB * w], BF16, tag="asing")
                    sv3 = sv[:, :].rearrange("p (b j) -> p b j", b=QB)
                    nc.scalar.copy(sv3[:, :, :], ps3[:, :, :w])
                    hv = stage.tile([H, QB * w2], BF16, tag="shalf")
                    hv3 = hv[:, :].rearrange("p (b j) -> p b j", b=QB)
                    nc.vector.tensor_tensor(
                        hv3[:, :, :], sv3[:, :, :w2], sv3[:, :, w2:w],
                        mybir.AluOpType.max,
                    )
                    nc.vector.reduce_max(
                        magg_sl(i0, QB), hv3[:, :, :],
                        axis=mybir.AxisListType.X,
                    )
            row0 += gsz
            if row0 == H and gsz == 32:
                tail_half(0)              # overlap first output half
        tail_half(1)

    nc.compile()
    nc.m = get_hw_module(nc.m)
    return nc


def _prepare(z, e_feat, adj, W_msg, b_msg, W_upd, b_upd):
    """Host-side sharding + compaction with per-block uniform widths.

    Rows sorted by active-edge count (desc); padding slots replicate the
    row's first active column (max-neutral).  Returns (in_maps, widths,
    orders); out rows come back permuted by `orders`.
    """
    W_i, W_j, W_e = W_msg[:Z], W_msg[Z : 2 * Z], W_msg[2 * Z :]
    Wu_z, Wu_h = W_upd[:Z], W_upd[Z:]

    counts = (adj > 0).sum(axis=-1)                   # [B, N]
    orders, csort = [], []
    for c in range(NCORES):
        b, half = divmod(c, NCORES // B)
        cnt = counts[b, half * IH : (half + 1) * IH]
        order = np.argsort(-cnt, kind="stable")
        orders.append(order)
        csort.append(cnt[order])
    csort = np.stack(csort)                           # [NCORES, IH]

    widths = []
    row0 = 0
    for gsz in SIZES:
        wmax = int(csort[:, row0 : row0 + gsz].max())
        widths.append(min(N, (wmax + 7) // 8 * 8))
        row0 += gsz
    widths = np.array(widths, dtype=int)
    offs = np.concatenate([[0], np.cumsum(np.array(SIZES) * widths)])
    tot = int(offs[-1])
    maxw = int(widths.max())

    W_aug = np.concatenate([W_e, W_j], axis=0).astype(NP_FP8)   # [80, H]
    lhst_np = np.ascontiguousarray(W_aug)
    wuh_np = np.ascontiguousarray(Wu_h, np.float32)
    ident_np = np.eye(H, dtype=np.float32)

    in_maps = []
    for c in range(NCORES):
        b, half = divmod(c, NCORES // B)
        sl = slice(half * IH, (half + 1) * IH)
        order = orders[c]
        adj_blk = (adj[b, sl] > 0)[order]             # [IH, N] sorted rows
        jorder = np.argsort(~adj_blk, axis=-1, kind="stable")[:, :maxw]
        pad = ~np.take_along_axis(adj_blk, jorder, axis=1)
        jorder = np.where(pad, jorder[:, :1], jorder)
        e_sel = np.take_along_axis(
            e_feat[b, sl][order], jorder[:, :, None], axis=1
        )                                             # [IH, maxw, E]
        z_sel = z[b][jorder]                          # [IH, maxw, Z]
        aug8 = np.concatenate(
            [e_sel.astype(NP_FP8), z_sel.astype(NP_FP8)], axis=-1
        )                                             # [IH, maxw, 80]

        stream = np.empty((KAUG, tot), dtype=NP_FP8)
        row0 = 0
        for blk, (gsz, w) in enumerate(zip(SIZES, widths)):
            blkv = aug8[row0 : row0 + gsz, :w, :]     # [gsz, w, 80]
            stream[:, offs[blk] : offs[blk + 1]] = blkv.transpose(2, 0, 1).reshape(
                KAUG, gsz * w
            )
            row0 += gsz

        zperm = z[b, sl][order]
        in_maps.append(
            {
                "stream": stream,
                "lhst": lhst_np,
                "zit": np.ascontiguousarray(
                    (zperm @ W_i).T + b_msg[:, None], dtype=np.float32
                ),
                "hostc": np.ascontiguousarray(
                    (zperm @ Wu_z + b_upd).T, dtype=np.float32
                ),
                "wuh": wuh_np,
                "ident": ident_np,
            }
        )
    return in_maps, widths, orders


def kernel(z, e_feat, adj, W_msg, b_msg, W_upd, b_upd):
    global LAST_RESULTS

    z = np.asarray(z, np.float32)
    e_feat = np.asarray(e_feat, np.float32)
    adj = np.asarray(adj)
    W_msg = np.asarray(W_msg, np.float32)
    b_msg = np.asarray(b_msg, np.float32)
    W_upd = np.asarray(W_upd, np.float32)
    b_upd = np.asarray(b_upd, np.float32)

    in_maps, widths, orders = _prepare(z, e_feat, adj, W_msg, b_msg, W_upd, b_upd)

    key = tuple(widths)
    if key not in _MODULE_CACHE:
        _MODULE_CACHE[key] = _build_module(widths)
    nc = _MODULE_CACHE[key]

    if TRACE:
        _ensure_ntff_hook()
    res = bass_utils.run_bass_kernel_spmd(
        nc, in_maps, core_ids=list(range(NCORES)), trace=TRACE, tmpdir=TRACE_DIR
    )
    LAST_RESULTS = res

    full = np.empty((B, N, H), np.float32)
    for c in range(NCORES):
        b, half = divmod(c, NCORES // B)
        full[b, half * IH + orders[c]] = res.results[c]["out"]
    return full


if __name__ == "__main__":
    rng = np.random.default_rng(0)
    ins = {
        "z": rng.standard_normal((B, N, Z)).astype(np.float32),
        "e_feat": rng.standard_normal((B, N, N, E)).astype(np.float32),
        "adj": (rng.random((B, N, N)) < 0.5).astype(np.int32),
        "W_msg": (rng.standard_normal((2 * Z + E, H)) * 0.1).astype(np.float32),
        "b_msg": np.zeros(H, np.float32),
        "W_upd": (rng.standard_normal((Z + H, H)) * 0.1).astype(np.float32),
        "b_upd": np.zeros(H, np.float32),
    }
    out = kernel(**ins)
    print("out", out.shape, out.dtype, float(np.abs(out).max()))


# revision 14
# speedup vs baseline: 1.1812x; 1.1304x over previous
"""Trainium2 Bass kernel for nn_MultiMPNN (gnn_message_passing).

Reference computation (B=4, N=512, Z=64, E=16, H=128):
    msgs[b,i,j,:] = z[b,i]@W_i + z[b,j]@W_j + e_feat[b,i,j]@W_e + b_msg
    agg[b,i,:]    = max_j (msgs + (adj>0 ? 0 : -inf))
    out           = z@Wu_z + agg@Wu_h + b_upd

Sharding: 8 cores = (batch b, half of destination rows i).  Each core owns
256 i-rows and the full j axis.

Device-side design (v3), tuned to measured TRN2 rates:
  PE matmul 1 col/cycle (ldweights hidden when back-to-back),
  DVE reduce 1.04 ns/elem (no fast mode), DVE tensor_tensor bf16 0.52
  ns/out (2x), ACT copy ~0.96 ns/elem, GPSIMD has no max op.

 1. One fp8 matmul per row folds everything under the max, augmented
    contraction K = E + Z = 80:
      lhsT[80,128] = [W_e ; W_j]  (const, e4m3)
      rhs [80,w]   = [e_feat[b,i,sel].T ; z[b,sel].T]  (streamed, e4m3)
    No mask lane: padding slots replicate the row's first ACTIVE column.
    zi + b_msg commute out of the max into the final linear (z@Wu_z and
    z@W_i computed on host, exact f32).
 2. Host compacts the j axis per row (only adj=1 columns), width uniform
    per block of count-sorted rows.
 3. PSUM drain per 4-row quad (row per bank), pattern a,a,d,a,a,d,a,a:
      d) DVE reduce_max straight from PSUM (1.04/elem, done in one).
      a) ACT copy PSUM->SBUF bf16; pairs of quads share a staging tile,
         then a DVE tensor_tensor(max) halving chain (w/2 -> w/4 -> w/8,
         0.26/elem consumed) + one small reduce_max finishes 8 rows.
 4. Filler matmuls into the unused tail columns of each quad's banks keep
    the PE p-state at 2.4 GHz (PE has slack; the drains are the
    bottleneck, and a cold PE would become one).
 5. The final linear runs in two i-halves so the first half overlaps the
    last quads.
"""

import numpy as np
import ml_dtypes

import concourse.bacc as bacc
import concourse.mybir as mybir
import concourse.tile as tile
from concourse import bass_utils
from concourse.bass_interp import get_hw_module
from contextlib import ExitStack

B, N, Z, E, H = 4, 512, 64, 16, 128
NCORES = 8
IH = N * B // NCORES          # 256 destination rows per core
KAUG = E + Z                  # 80
BANK = 512                    # f32 elems per PSUM bank
QB = 4                        # rows (banks) per quad

F32 = mybir.dt.float32
BF16 = mybir.dt.bfloat16
FP8 = mybir.dt.float8e4
NP_FP8 = ml_dtypes.float8_e4m3
NP_BF16 = ml_dtypes.bfloat16

SIZES = [8, 8, 8, 8] + [32] * 7
assert sum(SIZES) == IH
PAT_A = "daaaaaaa"            # per-quad paths in a 32-row block
PAT_RAMP = "da"               # 8-row ramp blocks
FILLERS = 0                   # PE p-state keeper matmuls per quad

TRACE = False
TRACE_DIR = None
LAST_RESULTS = None

_MODULE_CACHE = {}


def _ensure_ntff_hook():
    """The agent image's antenv lacks axon_hooks; recreate it so
    run_bass_kernel_spmd(trace=True) can reach the axon NTFF profiler."""
    import sys
    import types

    try:
        import antenv.axon_hooks  # noqa: F401

        return
    except ImportError:
        pass
    import antenv
    from trn_agent_boot.trn_boot import _ntff_profile_via_ctypes

    state = {"h": _ntff_profile_via_ctypes("/opt/axon/libaxon_pjrt.so")}
    mod = types.ModuleType("antenv.axon_hooks")
    mod.get_axon_ntff_profile_hook = lambda: state["h"]
    mod.set_axon_ntff_profile_hook = lambda h: state.__setitem__("h", h)
    sys.modules["antenv.axon_hooks"] = mod
    antenv.axon_hooks = mod


def _build_module(widths):
    widths = list(widths)
    offs = [0]
    for gsz, w in zip(SIZES, widths):
        offs.append(offs[-1] + gsz * w)
    tot = offs[-1]

    nc = bacc.Bacc(
        "TRN2",
        target_bir_lowering=False,
        debug=False,
        enable_asserts=False,
        num_devices=NCORES,
    )

    stream = nc.dram_tensor("stream", [KAUG, tot], FP8, kind="ExternalInput")
    lhst = nc.dram_tensor("lhst", [KAUG, H], FP8, kind="ExternalInput")
    zit = nc.dram_tensor("zit", [H, IH], F32, kind="ExternalInput")
    hostc = nc.dram_tensor("hostc", [H, IH], F32, kind="ExternalInput")
    wuh = nc.dram_tensor("wuh", [H, H], F32, kind="ExternalInput")
    ident = nc.dram_tensor("ident", [H, H], F32, kind="ExternalInput")
    out = nc.dram_tensor("out", [IH, H], F32, kind="ExternalOutput")

    with ExitStack() as ctx:
        tc = ctx.enter_context(tile.TileContext(nc))
        const = ctx.enter_context(tc.tile_pool(name="const", bufs=1))
        mega = ctx.enter_context(tc.tile_pool(name="mega", bufs=4))
        psum = ctx.enter_context(tc.tile_pool(name="psum", bufs=2, space="PSUM"))
        stage = ctx.enter_context(tc.tile_pool(name="stage", bufs=6))

        # stream block 0 + lhst first so the PE can start ASAP; big consts
        # go on the scalar DMA queue (needed only by the tail).
        stream_ap = stream.ap()
        mb0 = mega.tile([KAUG, SIZES[0] * widths[0]], FP8, tag="mega")
        nc.sync.dma_start(mb0[:, :], stream_ap[:, : offs[1]])
        lhst_sb = const.tile([KAUG, H], FP8, tag="lhst")
        nc.sync.dma_start(lhst_sb[:, :], lhst.ap())
        zit_sb = const.tile([H, IH], F32, tag="zit")
        nc.scalar.dma_start(zit_sb[:, :], zit.ap())
        hostc_sb = const.tile([H, IH], F32, tag="hostc")
        nc.scalar.dma_start(hostc_sb[:, :], hostc.ap())
        wuh_sb = const.tile([H, H], F32, tag="wuh")
        nc.scalar.dma_start(wuh_sb[:, :], wuh.ap())
        ident_sb = const.tile([H, H], F32, tag="ident")
        nc.scalar.dma_start(ident_sb[:, :], ident.ap())

        magg0 = const.tile([H, IH // 2], BF16, tag="magg0")
        magg1 = const.tile([H, IH // 2], BF16, tag="magg1")

        def magg_sl(i0, n):
            t, o = (magg0, i0) if i0 < IH // 2 else (magg1, i0 - IH // 2)
            return t[:, o : o + n]

        def tail_half(t):
            """Final linear + transpose + store for i-columns [t*128, t*128+128)."""
            sl = slice(t * H, (t + 1) * H)
            aggt = const.tile([H, H], F32, tag=f"aggt{t}")
            mt = magg0 if t == 0 else magg1
            nc.vector.tensor_add(aggt[:, :], mt[:, :], zit_sb[:, sl])
            psf = psum.tile([H, QB * BANK], F32, tag="ps")
            nc.tensor.matmul(
                psf[:, :H], wuh_sb[:, :], aggt[:, :], start=True, stop=True
            )
            outt = const.tile([H, H], F32, tag=f"outt{t}")
            nc.vector.tensor_add(outt[:, :], psf[:, :H], hostc_sb[:, sl])
            pst = psum.tile([H, QB * BANK], F32, tag="ps")
            nc.tensor.transpose(pst[:, :H], outt[:, :], ident_sb[:, :])
            osb = const.tile([H, H], F32, tag=f"osb{t}")
            nc.scalar.copy(osb[:, :], pst[:, :H])
            nc.sync.dma_start(out.ap()[sl, :], osb[:, :])

        row0 = 0
        for blk, (gsz, w) in enumerate(zip(SIZES, widths)):
            span = gsz * w
            off = offs[blk]
            if blk == 0:
                mb = mb0
            else:
                mb = mega.tile([KAUG, span], FP8, tag="mega")
                nc.sync.dma_start(mb[:, :], stream_ap[:, off : off + span])

            w2, w4, w8 = w // 2, w // 4, w // 8
            ftail = BANK - w                      # free tail cols per bank
            pat = PAT_RAMP if gsz < 32 else PAT_A
            nq = gsz // QB
            pend = None
            for q in range(nq):
                ps = psum.tile([H, QB * BANK], F32, tag="ps")
                ps3 = ps[:, :].rearrange("p (b j) -> p b j", b=QB)
                for r in range(QB):
                    c0 = (q * QB + r) * w
                    nc.tensor.matmul(
                        ps3[:, r, :w],
                        lhst_sb[:, :],
                        mb[:, c0 : c0 + w],
                        start=True,
                        stop=True,
                    )
                i0 = row0 + q * QB
                path = pat[q % len(pat)]
                if path == "d":
                    nc.vector.reduce_max(
                        magg_sl(i0, QB), ps3[:, :, :w],
                        axis=mybir.AxisListType.X,
                    )
                    continue
                nxt = pat[(q + 1) % len(pat)] if q + 1 < nq else "d"
                if pend is None and nxt == "a":
                    sv = stage.tile([H, 2 * QB * w], BF16, tag="apair")
                    sv4 = sv[:, :].rearrange("p (b j) -> p b j", b=2 * QB)
                    nc.scalar.copy(sv4[:, :QB, :], ps3[:, :, :w])
                    pend = (sv, i0)
                elif pend is not None:
                    sv, ip = pend
                    pend = None
                    assert ip + QB == i0
                    sv4 = sv[:, :].rearrange("p (b j) -> p b j", b=2 * QB)
                    nc.scalar.copy(sv4[:, QB:, :], ps3[:, :, :w])
                    hv = stage.tile([H, 2 * QB * w2], BF16, tag="ahalf")
                    hv4 = hv[:, :].rearrange("p (b j) -> p b j", b=2 * QB)
                    nc.vector.tensor_tensor(
                        hv4[:, :, :], sv4[:, :, :w2], sv4[:, :, w2:w],
                        mybir.AluOpType.max,
                    )
                    qv = stage.tile([H, 2 * QB * w4], BF16, tag="aquar")
                    qv4 = qv[:, :].rearrange("p (b j) -> p b j", b=2 * QB)
                    nc.vector.tensor_tensor(
                        qv4[:, :, :], hv4[:, :, :w4], hv4[:, :, w4:w2],
                        mybir.AluOpType.max,
                    )
                    nc.vector.reduce_max(
                        magg_sl(ip, 2 * QB), qv4[:, :, :],
                        axis=mybir.AxisListType.X,
                    )
                else:  # single 'a' quad (ramp blocks)
                    sv = stage.tile([H, QB * w], BF16, tag="asing")
                    sv3 = sv[:, :].rearrange("p (b j) -> p b j", b=QB)
                    nc.scalar.copy(sv3[:, :, :], ps3[:, :, :w])
                    hv = stage.tile([H, QB * w2], BF16, tag="shalf")
                    hv3 = hv[:, :].rearrange("p (b j) -> p b j", b=QB)
                    nc.vector.tensor_tensor(
                        hv3[:, :, :], sv3[:, :, :w2], sv3[:, :, w2:w],
                        mybir.AluOpType.max,
                    )
                    qs = stage.tile([H, QB * w4], BF16, tag="squar")
                    qs3 = qs[:, :].rearrange("p (b j) -> p b j", b=QB)
                    nc.vector.tensor_tensor(
                        qs3[:, :, :], hv3[:, :, :w4], hv3[:, :, w4:w2],
                        mybir.AluOpType.max,
                    )
                    nc.vector.reduce_max(
                        magg_sl(i0, QB), qs3[:, :, :],
                        axis=mybir.AxisListType.X,
                    )
            row0 += gsz
            if row0 == H and gsz == 32:
                tail_half(0)              # overlap first output half
        tail_half(1)

    nc.compile()
    nc.m = get_hw_module(nc.m)
    return nc


def _prepare(z, e_feat, adj, W_msg, b_msg, W_upd, b_upd):
    """Host-side sharding + compaction with per-block uniform widths.

    Rows sorted by active-edge count (desc); padding slots replicate the
    row's first active column (max-neutral).  Returns (in_maps, widths,
    orders); out rows come back permuted by `orders`.
    """
    W_i, W_j, W_e = W_msg[:Z], W_msg[Z : 2 * Z], W_msg[2 * Z :]
    Wu_z, Wu_h = W_upd[:Z], W_upd[Z:]

    counts = (adj > 0).sum(axis=-1)                   # [B, N]
    orders, csort = [], []
    for c in range(NCORES):
        b, half = divmod(c, NCORES // B)
        cnt = counts[b, half * IH : (half + 1) * IH]
        order = np.argsort(-cnt, kind="stable")
        orders.append(order)
        csort.append(cnt[order])
    csort = np.stack(csort)                           # [NCORES, IH]

    widths = []
    row0 = 0
    for gsz in SIZES:
        wmax = int(csort[:, row0 : row0 + gsz].max())
        widths.append(min(N, (wmax + 7) // 8 * 8))
        row0 += gsz
    widths = np.array(widths, dtype=int)
    offs = np.concatenate([[0], np.cumsum(np.array(SIZES) * widths)])
    tot = int(offs[-1])
    maxw = int(widths.max())

    W_aug = np.concatenate([W_e, W_j], axis=0).astype(NP_FP8)   # [80, H]
    lhst_np = np.ascontiguousarray(W_aug)
    wuh_np = np.ascontiguousarray(Wu_h, np.float32)
    ident_np = np.eye(H, dtype=np.float32)

    in_maps = []
    for c in range(NCORES):
        b, half = divmod(c, NCORES // B)
        sl = slice(half * IH, (half + 1) * IH)
        order = orders[c]
        adj_blk = (adj[b, sl] > 0)[order]             # [IH, N] sorted rows
        jorder = np.argsort(~adj_blk, axis=-1, kind="stable")[:, :maxw]
        pad = ~np.take_along_axis(adj_blk, jorder, axis=1)
        jorder = np.where(pad, jorder[:, :1], jorder)
        e_sel = np.take_along_axis(
            e_feat[b, sl][order], jorder[:, :, None], axis=1
        )                                             # [IH, maxw, E]
        z_sel = z[b][jorder]                          # [IH, maxw, Z]
        aug8 = np.concatenate(
            [e_sel.astype(NP_FP8), z_sel.astype(NP_FP8)], axis=-1
        )                                             # [IH, maxw, 80]

        stream = np.empty((KAUG, tot), dtype=NP_FP8)
        row0 = 0
        for blk, (gsz, w) in enumerate(zip(SIZES, widths)):
            blkv = aug8[row0 : row0 + gsz, :w, :]     # [gsz, w, 80]
            stream[:, offs[blk] : offs[blk + 1]] = blkv.transpose(2, 0, 1).reshape(
                KAUG, gsz * w
            )
            row0 += gsz

        zperm = z[b, sl][order]
        in_maps.append(
            {
                "stream": stream,
                "lhst": lhst_np,
                "zit": np.ascontiguousarray(
                    (zperm @ W_i).T + b_msg[:, None], dtype=np.float32
                ),
                "hostc": np.ascontiguousarray(
                    (zperm @ Wu_z + b_upd).T, dtype=np.float32
                ),
                "wuh": wuh_np,
                "ident": ident_np,
            }
        )
    return in_maps, widths, orders


def kernel(z, e_feat, adj, W_msg, b_msg, W_upd, b_upd):
    global LAST_RESULTS

    z = np.asarray(z, np.float32)
    e_feat = np.asarray(e_feat, np.float32)
    adj = np.asarray(adj)
    W_msg = np.asarray(W_msg, np.float32)
    b_msg = np.asarray(b_msg, np.float32)
    W_upd = np.asarray(W_upd, np.float32)
    b_upd = np.asarray(b_upd, np.float32)

    in_maps, widths, orders = _prepare(z, e_feat, adj, W_msg, b_msg, W_upd, b_upd)

    key = tuple(widths)
    if key not in _MODULE_CACHE:
        _MODULE_CACHE[key] = _build_module(widths)
    nc = _MODULE_CACHE[key]

    if TRACE:
        _ensure_ntff_hook()
    res = bass_utils.run_bass_kernel_spmd(
        nc, in_maps, core_ids=list(range(NCORES)), trace=TRACE, tmpdir=TRACE_DIR
    )
    LAST_RESULTS = res

    full = np.empty((B, N, H), np.float32)
    for c in range(NCORES):
        b, half = divmod(c, NCORES // B)
        full[b, half * IH + orders[c]] = res.results[c]["out"]
    return full


if __name__ == "__main__":
    rng = np.random.default_rng(0)
    ins = {
        "z": rng.standard_normal((B, N, Z)).astype(np.float32),
        "e_feat": rng.standard_normal((B, N, N, E)).astype(np.float32),
        "adj": (rng.random((B, N, N)) < 0.5).astype(np.int32),
        "W_msg": (rng.standard_normal((2 * Z + E, H)) * 0.1).astype(np.float32),
        "b_msg": np.zeros(H, np.float32),
        "W_upd": (rng.standard_normal((Z + H, H)) * 0.1).astype(np.float32),
        "b_upd": np.zeros(H, np.float32),
    }
    out = kernel(**ins)
    print("out", out.shape, out.dtype, float(np.abs(out).max()))


# revision 16
# speedup vs baseline: 1.2081x; 1.0228x over previous
"""Trainium2 Bass kernel for nn_MultiMPNN (gnn_message_passing).

Reference computation (B=4, N=512, Z=64, E=16, H=128):
    msgs[b,i,j,:] = z[b,i]@W_i + z[b,j]@W_j + e_feat[b,i,j]@W_e + b_msg
    agg[b,i,:]    = max_j (msgs + (adj>0 ? 0 : -inf))
    out           = z@Wu_z + agg@Wu_h + b_upd

Sharding: 8 cores = (batch b, half of destination rows i).  Each core owns
256 i-rows and the full j axis.

Device-side design (v3), tuned to measured TRN2 rates:
  PE matmul 1 col/cycle (ldweights hidden when back-to-back),
  DVE reduce 1.04 ns/elem (no fast mode), DVE tensor_tensor bf16 0.52
  ns/out (2x), ACT copy ~0.96 ns/elem, GPSIMD has no max op.

 1. One fp8 matmul per row folds everything under the max, augmented
    contraction K = E + Z = 80:
      lhsT[80,128] = [W_e ; W_j]  (const, e4m3)
      rhs [80,w]   = [e_feat[b,i,sel].T ; z[b,sel].T]  (streamed, e4m3)
    No mask lane: padding slots replicate the row's first ACTIVE column.
    zi + b_msg commute out of the max into the final linear (z@Wu_z and
    z@W_i computed on host, exact f32).
 2. Host compacts the j axis per row (only adj=1 columns), width uniform
    per block of count-sorted rows.
 3. PSUM drain per 4-row quad (row per bank), pattern a,a,d,a,a,d,a,a:
      d) DVE reduce_max straight from PSUM (1.04/elem, done in one).
      a) ACT copy PSUM->SBUF bf16; pairs of quads share a staging tile,
         then a DVE tensor_tensor(max) halving chain (w/2 -> w/4 -> w/8,
         0.26/elem consumed) + one small reduce_max finishes 8 rows.
 4. Filler matmuls into the unused tail columns of each quad's banks keep
    the PE p-state at 2.4 GHz (PE has slack; the drains are the
    bottleneck, and a cold PE would become one).
 5. The final linear runs in two i-halves so the first half overlaps the
    last quads.
"""

import numpy as np
import ml_dtypes

import concourse.bacc as bacc
import concourse.mybir as mybir
import concourse.tile as tile
from concourse import bass_utils
from concourse.bass_interp import get_hw_module
from contextlib import ExitStack

B, N, Z, E, H = 4, 512, 64, 16, 128
NCORES = 8
IH = N * B // NCORES          # 256 destination rows per core
KAUG = E + Z                  # 80
BANK = 512                    # f32 elems per PSUM bank
QB = 4                        # rows (banks) per quad

F32 = mybir.dt.float32
BF16 = mybir.dt.bfloat16
FP8 = mybir.dt.float8e4
NP_FP8 = ml_dtypes.float8_e4m3
NP_BF16 = ml_dtypes.bfloat16

SIZES = [8, 24] + [32] * 7
assert sum(SIZES) == IH
PAT_A = "daaaaaaa"            # per-quad paths in a 32-row block
PAT_RAMP = "da"               # 8-row ramp blocks
FILLERS = 0                   # PE p-state keeper matmuls per quad

TRACE = False
TRACE_DIR = None
LAST_RESULTS = None

_MODULE_CACHE = {}


def _ensure_ntff_hook():
    """The agent image's antenv lacks axon_hooks; recreate it so
    run_bass_kernel_spmd(trace=True) can reach the axon NTFF profiler."""
    import sys
    import types

    try:
        import antenv.axon_hooks  # noqa: F401

        return
    except ImportError:
        pass
    import antenv
    from trn_agent_boot.trn_boot import _ntff_profile_via_ctypes

    state = {"h": _ntff_profile_via_ctypes("/opt/axon/libaxon_pjrt.so")}
    mod = types.ModuleType("antenv.axon_hooks")
    mod.get_axon_ntff_profile_hook = lambda: state["h"]
    mod.set_axon_ntff_profile_hook = lambda h: state.__setitem__("h", h)
    sys.modules["antenv.axon_hooks"] = mod
    antenv.axon_hooks = mod


def _build_module(widths):
    widths = list(widths)
    offs = [0]
    for gsz, w in zip(SIZES, widths):
        offs.append(offs[-1] + gsz * w)
    tot = offs[-1]

    nc = bacc.Bacc(
        "TRN2",
        target_bir_lowering=False,
        debug=False,
        enable_asserts=False,
        num_devices=NCORES,
    )

    stream = nc.dram_tensor("stream", [KAUG, tot], FP8, kind="ExternalInput")
    lhst = nc.dram_tensor("lhst", [KAUG, H], FP8, kind="ExternalInput")
    zit = nc.dram_tensor("zit", [H, IH], F32, kind="ExternalInput")
    hostc = nc.dram_tensor("hostc", [H, IH], F32, kind="ExternalInput")
    wuh = nc.dram_tensor("wuh", [H, H], F32, kind="ExternalInput")
    ident = nc.dram_tensor("ident", [H, H], F32, kind="ExternalInput")
    out = nc.dram_tensor("out", [IH, H], F32, kind="ExternalOutput")

    with ExitStack() as ctx:
        tc = ctx.enter_context(tile.TileContext(nc))
        const = ctx.enter_context(tc.tile_pool(name="const", bufs=1))
        mega = ctx.enter_context(tc.tile_pool(name="mega", bufs=4))
        psum = ctx.enter_context(tc.tile_pool(name="psum", bufs=2, space="PSUM"))
        stage = ctx.enter_context(tc.tile_pool(name="stage", bufs=6))

        # stream block 0 + lhst first so the PE can start ASAP; big consts
        # go on the scalar DMA queue (needed only by the tail).
        stream_ap = stream.ap()
        mb0 = mega.tile([KAUG, SIZES[0] * widths[0]], FP8, tag="mega")
        nc.sync.dma_start(mb0[:, :], stream_ap[:, : offs[1]])
        lhst_sb = const.tile([KAUG, H], FP8, tag="lhst")
        nc.sync.dma_start(lhst_sb[:, :], lhst.ap())
        zit_sb = const.tile([H, IH], F32, tag="zit")
        nc.scalar.dma_start(zit_sb[:, :], zit.ap())
        hostc_sb = const.tile([H, IH], F32, tag="hostc")
        nc.scalar.dma_start(hostc_sb[:, :], hostc.ap())
        wuh_sb = const.tile([H, H], F32, tag="wuh")
        nc.scalar.dma_start(wuh_sb[:, :], wuh.ap())
        ident_sb = const.tile([H, H], F32, tag="ident")
        nc.scalar.dma_start(ident_sb[:, :], ident.ap())

        magg0 = const.tile([H, IH // 2], BF16, tag="magg0")
        magg1 = const.tile([H, IH // 2], BF16, tag="magg1")

        def magg_sl(i0, n):
            t, o = (magg0, i0) if i0 < IH // 2 else (magg1, i0 - IH // 2)
            return t[:, o : o + n]

        def tail_half(t):
            """Final linear + transpose + store for i-columns [t*128, t*128+128)."""
            sl = slice(t * H, (t + 1) * H)
            aggt = const.tile([H, H], F32, tag=f"aggt{t}")
            mt = magg0 if t == 0 else magg1
            nc.vector.tensor_add(aggt[:, :], mt[:, :], zit_sb[:, sl])
            psf = psum.tile([H, QB * BANK], F32, tag="ps")
            nc.tensor.matmul(
                psf[:, :H], wuh_sb[:, :], aggt[:, :], start=True, stop=True
            )
            outt = const.tile([H, H], F32, tag=f"outt{t}")
            nc.vector.tensor_add(outt[:, :], psf[:, :H], hostc_sb[:, sl])
            pst = psum.tile([H, QB * BANK], F32, tag="ps")
            nc.tensor.transpose(pst[:, :H], outt[:, :], ident_sb[:, :])
            osb = const.tile([H, H], F32, tag=f"osb{t}")
            nc.scalar.copy(osb[:, :], pst[:, :H])
            nc.sync.dma_start(out.ap()[sl, :], osb[:, :])

        row0 = 0
        for blk, (gsz, w) in enumerate(zip(SIZES, widths)):
            span = gsz * w
            off = offs[blk]
            if blk == 0:
                mb = mb0
            else:
                mb = mega.tile([KAUG, span], FP8, tag="mega")
                nc.sync.dma_start(mb[:, :], stream_ap[:, off : off + span])

            w2, w4, w8 = w // 2, w // 4, w // 8
            ftail = BANK - w                      # free tail cols per bank
            pat = PAT_RAMP if gsz <= 8 else PAT_A
            nq = gsz // QB
            pend = None
            for q in range(nq):
                ps = psum.tile([H, QB * BANK], F32, tag="ps")
                ps3 = ps[:, :].rearrange("p (b j) -> p b j", b=QB)
                for r in range(QB):
                    c0 = (q * QB + r) * w
                    nc.tensor.matmul(
                        ps3[:, r, :w],
                        lhst_sb[:, :],
                        mb[:, c0 : c0 + w],
                        start=True,
                        stop=True,
                    )
                i0 = row0 + q * QB
                path = pat[q % len(pat)]
                if path == "d":
                    nc.vector.reduce_max(
                        magg_sl(i0, QB), ps3[:, :, :w],
                        axis=mybir.AxisListType.X,
                    )
                    continue
                nxt = pat[(q + 1) % len(pat)] if q + 1 < nq else "d"
                if pend is None and nxt == "a":
                    sv = stage.tile([H, 2 * QB * w], BF16, tag="apair")
                    sv4 = sv[:, :].rearrange("p (b j) -> p b j", b=2 * QB)
                    nc.scalar.copy(sv4[:, :QB, :], ps3[:, :, :w])
                    pend = (sv, i0)
                elif pend is not None:
                    sv, ip = pend
                    pend = None
                    assert ip + QB == i0
                    sv4 = sv[:, :].rearrange("p (b j) -> p b j", b=2 * QB)
                    nc.scalar.copy(sv4[:, QB:, :], ps3[:, :, :w])
                    hv = stage.tile([H, 2 * QB * w2], BF16, tag="ahalf")
                    hv4 = hv[:, :].rearrange("p (b j) -> p b j", b=2 * QB)
                    nc.vector.tensor_tensor(
                        hv4[:, :, :], sv4[:, :, :w2], sv4[:, :, w2:w],
                        mybir.AluOpType.max,
                    )
                    qv = stage.tile([H, 2 * QB * w4], BF16, tag="aquar")
                    qv4 = qv[:, :].rearrange("p (b j) -> p b j", b=2 * QB)
                    nc.vector.tensor_tensor(
                        qv4[:, :, :], hv4[:, :, :w4], hv4[:, :, w4:w2],
                        mybir.AluOpType.max,
                    )
                    nc.vector.reduce_max(
                        magg_sl(ip, 2 * QB), qv4[:, :, :],
                        axis=mybir.AxisListType.X,
                    )
                else:  # single 'a' quad (ramp blocks)
                    sv = stage.tile([H, QB * w], BF16, tag="asing")
                    sv3 = sv[:, :].rearrange("p (b j) -> p b j", b=QB)
                    nc.scalar.copy(sv3[:, :, :], ps3[:, :, :w])
                    hv = stage.tile([H, QB * w2], BF16, tag="shalf")
                    hv3 = hv[:, :].rearrange("p (b j) -> p b j", b=QB)
                    nc.vector.tensor_tensor(
                        hv3[:, :, :], sv3[:, :, :w2], sv3[:, :, w2:w],
                        mybir.AluOpType.max,
                    )
                    qs = stage.tile([H, QB * w4], BF16, tag="squar")
                    qs3 = qs[:, :].rearrange("p (b j) -> p b j", b=QB)
                    nc.vector.tensor_tensor(
                        qs3[:, :, :], hv3[:, :, :w4], hv3[:, :, w4:w2],
                        mybir.AluOpType.max,
                    )
                    nc.vector.reduce_max(
                        magg_sl(i0, QB), qs3[:, :, :],
                        axis=mybir.AxisListType.X,
                    )
            row0 += gsz
        tail_half(0)
        tail_half(1)

    nc.compile()
    nc.m = get_hw_module(nc.m)
    return nc


def _prepare(z, e_feat, adj, W_msg, b_msg, W_upd, b_upd):
    """Host-side sharding + compaction with per-block uniform widths.

    Rows sorted by active-edge count (desc); padding slots replicate the
    row's first active column (max-neutral).  Returns (in_maps, widths,
    orders); out rows come back permuted by `orders`.
    """
    W_i, W_j, W_e = W_msg[:Z], W_msg[Z : 2 * Z], W_msg[2 * Z :]
    Wu_z, Wu_h = W_upd[:Z], W_upd[Z:]

    counts = (adj > 0).sum(axis=-1)                   # [B, N]
    orders, csort = [], []
    for c in range(NCORES):
        b, half = divmod(c, NCORES // B)
        cnt = counts[b, half * IH : (half + 1) * IH]
        order = np.argsort(-cnt, kind="stable")
        orders.append(order)
        csort.append(cnt[order])
    csort = np.stack(csort)                           # [NCORES, IH]

    widths = []
    row0 = 0
    for gsz in SIZES:
        wmax = int(csort[:, row0 : row0 + gsz].max())
        widths.append(min(N, (wmax + 7) // 8 * 8))
        row0 += gsz
    widths = np.array(widths, dtype=int)
    offs = np.concatenate([[0], np.cumsum(np.array(SIZES) * widths)])
    tot = int(offs[-1])
    maxw = int(widths.max())

    W_aug = np.concatenate([W_e, W_j], axis=0).astype(NP_FP8)   # [80, H]
    lhst_np = np.ascontiguousarray(W_aug)
    wuh_np = np.ascontiguousarray(Wu_h, np.float32)
    ident_np = np.eye(H, dtype=np.float32)

    in_maps = []
    for c in range(NCORES):
        b, half = divmod(c, NCORES // B)
        sl = slice(half * IH, (half + 1) * IH)
        order = orders[c]
        adj_blk = (adj[b, sl] > 0)[order]             # [IH, N] sorted rows
        jorder = np.argsort(~adj_blk, axis=-1, kind="stable")[:, :maxw]
        pad = ~np.take_along_axis(adj_blk, jorder, axis=1)
        jorder = np.where(pad, jorder[:, :1], jorder)
        e_sel = np.take_along_axis(
            e_feat[b, sl][order], jorder[:, :, None], axis=1
        )                                             # [IH, maxw, E]
        z_sel = z[b][jorder]                          # [IH, maxw, Z]
        aug8 = np.concatenate(
            [e_sel.astype(NP_FP8), z_sel.astype(NP_FP8)], axis=-1
        )                                             # [IH, maxw, 80]

        stream = np.empty((KAUG, tot), dtype=NP_FP8)
        row0 = 0
        for blk, (gsz, w) in enumerate(zip(SIZES, widths)):
            blkv = aug8[row0 : row0 + gsz, :w, :]     # [gsz, w, 80]
            stream[:, offs[blk] : offs[blk + 1]] = blkv.transpose(2, 0, 1).reshape(
                KAUG, gsz * w
            )
            row0 += gsz

        zperm = z[b, sl][order]
        in_maps.append(
            {
                "stream": stream,
                "lhst": lhst_np,
                "zit": np.ascontiguousarray(
                    (zperm @ W_i).T + b_msg[:, None], dtype=np.float32
                ),
                "hostc": np.ascontiguousarray(
                    (zperm @ Wu_z + b_upd).T, dtype=np.float32
                ),
                "wuh": wuh_np,
                "ident": ident_np,
            }
        )
    return in_maps, widths, orders


def kernel(z, e_feat, adj, W_msg, b_msg, W_upd, b_upd):
    global LAST_RESULTS

    z = np.asarray(z, np.float32)
    e_feat = np.asarray(e_feat, np.float32)
    adj = np.asarray(adj)
    W_msg = np.asarray(W_msg, np.float32)
    b_msg = np.asarray(b_msg, np.float32)
    W_upd = np.asarray(W_upd, np.float32)
    b_upd = np.asarray(b_upd, np.float32)

    in_maps, widths, orders = _prepare(z, e_feat, adj, W_msg, b_msg, W_upd, b_upd)

    key = tuple(widths)
    if key not in _MODULE_CACHE:
        _MODULE_CACHE[key] = _build_module(widths)
    nc = _MODULE_CACHE[key]

    if TRACE:
        _ensure_ntff_hook()
    res = bass_utils.run_bass_kernel_spmd(
        nc, in_maps, core_ids=list(range(NCORES)), trace=TRACE, tmpdir=TRACE_DIR
    )
    LAST_RESULTS = res

    full = np.empty((B, N, H), np.float32)
    for c in range(NCORES):
        b, half = divmod(c, NCORES // B)
        full[b, half * IH + orders[c]] = res.results[c]["out"]
    return full


if __name__ == "__main__":
    rng = np.random.default_rng(0)
    ins = {
        "z": rng.standard_normal((B, N, Z)).astype(np.float32),
        "e_feat": rng.standard_normal((B, N, N, E)).astype(np.float32),
        "adj": (rng.random((B, N, N)) < 0.5).astype(np.int32),
        "W_msg": (rng.standard_normal((2 * Z + E, H)) * 0.1).astype(np.float32),
        "b_msg": np.zeros(H, np.float32),
        "W_upd": (rng.standard_normal((Z + H, H)) * 0.1).astype(np.float32),
        "b_upd": np.zeros(H, np.float32),
    }
    out = kernel(**ins)
    print("out", out.shape, out.dtype, float(np.abs(out).max()))
